# revision 9
# baseline (speedup 1.0000x reference)
"""DiM block (Mamba-style selective-scan transformer block) on 8 TRN2 cores.

Sharding: core i handles (b = i//4, k = i%4). Spatial permutation q_k is
host-prepared so ONE SPMD program serves all 8 cores. Per-sample combine
over k via 4 chunked AllGathers (one per 128-row d-block of ys), overlapped
with the scan (d-block-outer loops, n-range split in two halves so the B/C
row-broadcast tiles load once per half).

Scan tile: partitions = (n-pair:2, d:64), free = t. Per (j, n-half)
section: PE broadcasts -dt into pinned PSUM (selector matmul), ACT applies
exp(A*dt) via per-partition A columns, DVE runs both elementwise muls (bf16
2x), the hw scan runs 7/8 on GPSIMD / 1/8 on DVE, PE reduces n-pairs into a
per-j PSUM accumulator seeded with Dp*xs (host-scaled selector).

LN stats everywhere use PE ones-matmuls on transposed tiles to produce
(128,L) per-token broadcast tiles directly (no transpose/DMA-broadcast
chains). The post phase folds LN-y into three Wout passes (T1 = Wl@(y*sz),
T3 = Wl@sz, T2 = Wlnb@sz) so only pointwise work remains after the last
collective. All weights arrive host-packed bf16, one DMA each.
"""
import json
import sys

sys.path.insert(0, "/opt/trn_rl_repo")

import numpy as np
import concourse.bass as bass
import concourse.mybir as mybir
import concourse.tile as tile
from concourse.bass_utils import run_bass_kernel_spmd

# ---------------------------------------------------------------------------
# Workaround: this walrus build rejects instructions carrying >1 embedded
# sem-wait. Split extra waits onto same-engine NoOps at BIR serialization.
_MAXW = 1
_wsplit_counter = [0]


def _split_multi_waits(bir: dict) -> dict:
    for fn in bir.get("functions", []):
        for bb in fn.get("blocks", []):
            insts = bb.get("instructions", [])
            if not any(
                len((i.get("sync_info") or {}).get("on_wait") or []) > _MAXW
                for i in insts
            ):
                continue
            out = []
            for inst in insts:
                si = inst.get("sync_info")
                waits = (si or {}).get("on_wait") or []
                if len(waits) > _MAXW and inst.get("engine"):
                    for w in waits[:-_MAXW]:
                        _wsplit_counter[0] += 1
                        out.append({
                            "debug": inst.get("debug", 0),
                            "engine": inst["engine"],
                            "ins": [], "outs": [],
                            "name": f"I-wsplit-{_wsplit_counter[0]}",
                            "opcode": "NoOp",
                            "sync_info": {"on_update": [], "on_wait": [w]},
                        })
                    si["on_wait"] = waits[-_MAXW:]
                out.append(inst)
            bb["instructions"] = out
    return bir


_orig_to_json_bytes = bass.Bass.to_json_bytes


def _patched_to_json_bytes(self) -> bytes:
    j = json.loads(_orig_to_json_bytes(self))
    _split_multi_waits(j)
    return json.dumps(j).encode()


bass.Bass.to_json_bytes = _patched_to_json_bytes

# ---------------------------------------------------------------------------
B, Hs, Ws, DIM = 2, 32, 32, 256
L = Hs * Ws
DI = 2 * DIM
DS = 64
DTR = DIM // 16
K = 4
HID = 4 * DIM

f32 = mybir.dt.float32
bf16 = mybir.dt.bfloat16
MUL = mybir.AluOpType.mult
ADD = mybir.AluOpType.add
SUB = mybir.AluOpType.subtract
BYP = mybir.AluOpType.bypass
AF = mybir.ActivationFunctionType
AX = mybir.AxisListType

EPS = 1e-6
NPAIRS = DS // 2

SP_OFF = {"b_in_xi": 0, "b_in_z": 4, "convb": 8, "dtb": 12,
          "lnw_c": 16, "lnb_c": 20, "b_out_c": 24, "b_fc1_c": 26,
          "b_fc2_c": 34}


def build_program(scan_dve_mod=6, pipe_depth=2, ycmod=7):
    nc = bass.Bass()

    def din(name, shape, dt=f32):
        return nc.dram_tensor(name, list(shape), dt, kind="ExternalInput")

    T = {}
    T["xT_pre_b"] = din("xT_pre_b", (128, 2 * L), bf16)
    T["xT_row_b"] = din("xT_row_b", (128, 2 * L), bf16)
    T["W_in_xi"] = din("W_in_xi", (128, 2 * DI), bf16)
    T["W_in_z"] = din("W_in_z", (128, 2 * DI), bf16)
    T["convdiag"] = din("convdiag", (128, 36 * 128), bf16)
    T["W_xp"] = din("W_xp", (128, 4 * 144), bf16)  # cols [B(64), C(64), dtr]
    T["W_dtm"] = din("W_dtm", (DTR, DI), bf16)
    T["acols"] = din("acols", (128, 256))
    T["sel2n"] = din("sel2n", (128, 2 * 128), bf16)
    T["ysel"] = din("ysel", (128, 64), bf16)
    T["dpsel"] = din("dpsel", (128, 512), bf16)
    T["W_out"] = din("W_out", (128, 4 * DIM), bf16)
    T["W_outb"] = din("W_outb", (128, 4 * DIM), bf16)
    T["W_fc1"] = din("W_fc1", (128, 2 * HID), bf16)
    T["W_fc2"] = din("W_fc2", (128, 8 * DIM), bf16)
    T["smallpack"] = din("smallpack", (128, 56))

    T["outT"] = nc.dram_tensor("outT", [DIM, L], bf16, kind="ExternalOutput")
    T["ys_lb"] = nc.dram_tensor("ys_lb", [DI, L], bf16)
    # gathered ys: three 128-row chunks + two 64-row chunks (tail-sized)
    T["ys_gb3"] = nc.dram_tensor("ys_gb3", [3, 4, 128, L], bf16)
    T["ys_gbt"] = nc.dram_tensor("ys_gbt", [2, 4, 64, L], bf16)

    with tile.TileContext(nc) as tc:
        _build_body(nc, tc, T, scan_dve_mod, pipe_depth, ycmod)
    return nc


def _build_body(nc, tc, T, scan_dve_mod, pipe_depth, ycmod=6):
    from contextlib import ExitStack

    dma = nc.sync.dma_start
    dma_act = nc.scalar.dma_start
    dma_gps = nc.gpsimd.dma_start

    perstack = ExitStack()
    persist = perstack.enter_context(tc.tile_pool(name="persist", bufs=1))
    wstack = ExitStack()
    wp = wstack.enter_context(tc.tile_pool(name="weights", bufs=1))
    prestack = ExitStack()
    work = prestack.enter_context(tc.tile_pool(name="prew", bufs=1))
    pre_ps = ExitStack()
    pp = pre_ps.enter_context(tc.tile_pool(name="ps_pre", bufs=2, space="PSUM"))

    # ---------------- S0: single-DMA packed loads -------------------------
    def load1(dname, cols, pool, dt=bf16, name=None, q=dma):
        t = pool.tile([128, cols], dt, tag=name or dname, name=name or dname)
        q(t[:], T[dname][:, :])
        return t

    xTpre_t = load1("xT_pre_b", 2 * L, work)
    xTrow_t = load1("xT_row_b", 2 * L, work)
    spk = persist.tile([128, 56], f32, tag="smallpack", name="smallpack")
    dma(spk[:], T["smallpack"][:, :])
    acols = persist.tile([128, 256], f32, tag="acols", name="acols")
    dma(acols[:], T["acols"][:, :])

    Wxi_t = load1("W_in_xi", 2 * DI, work, q=dma_act)
    Wz_t = load1("W_in_z", 2 * DI, work, q=dma_act)
    cdiag_t = load1("convdiag", 36 * 128, work, q=dma_act)
    Wxp_t = load1("W_xp", 4 * 144, work, q=dma_act)
    sel_t = load1("sel2n", 2 * 128, wp, q=dma_act)
    ysel_b = load1("ysel", 64, wp, q=dma_act)
    dpsel_b = load1("dpsel", 512, wp, q=dma_act)
    Wdt = wp.tile([DTR, DI], bf16, tag="Wdt", name="Wdt")
    dma_act(Wdt[:], T["W_dtm"][:, :])

    def sml(nm, i):
        o = SP_OFF[nm]
        return spk[:, o + i:o + i + 1]

    eps_col = persist.tile([128, 1], f32, tag="eps_col", name="eps_col")
    nc.gpsimd.memset(eps_col[:], EPS)
    oD256 = persist.tile([128, 128], bf16, tag="oD256", name="oD256")
    nc.gpsimd.memset(oD256[:], 1.0 / DIM)
    oD512 = persist.tile([128, 128], bf16, tag="oD512", name="oD512")
    nc.gpsimd.memset(oD512[:], 1.0 / DI)

    # adaLN modulation arrives host-computed in smallpack[36:56]
    mcols = [spk[:, 36 + 2 * i6:36 + 2 * i6 + 2] for i6 in range(6)]
    sh_msa, sc_msa, g_msa, sh_mlp, sc_mlp, g_mlp = mcols
    s1_msa = spk[:, 48:50]
    s1_mlp = spk[:, 50:52]
    gb_out = spk[:, 52:54]
    gb_fc2 = spk[:, 54:56]

    # ---------------- S2: LN1 + modulate via PE ones-broadcast ------------
    def ln_affine_T(xTt, shc, s1c, name):
        pmu = pp.tile([128, L], f32, tag="projp", name=f"pmu_{name}", bufs=2)
        psq = pp.tile([128, L], f32, tag="projp", name=f"psq_{name}", bufs=2)
        xsq = []
        for cc in range(2):
            sqt = work.tile([128, L], bf16, tag="xsqT", name=f"xsq_{name}{cc}", bufs=2)
            nc.vector.tensor_tensor(sqt[:], xTt[:, cc * L:(cc + 1) * L],
                                    xTt[:, cc * L:(cc + 1) * L], MUL)
            xsq.append(sqt)
        for cc in range(2):
            for th in range(2):
                nc.tensor.matmul(pmu[:, th * 512:(th + 1) * 512], oD256[:],
                                 xTt[:, cc * L + th * 512:cc * L + (th + 1) * 512],
                                 start=(cc == 0), stop=(cc == 1))
                nc.tensor.matmul(psq[:, th * 512:(th + 1) * 512], oD256[:],
                                 xsq[cc][:, th * 512:(th + 1) * 512],
                                 start=(cc == 0), stop=(cc == 1))
        mu_b = work.tile([128, L], bf16, tag=f"mub_{name}", name=f"mub_{name}")
        nc.scalar.copy(mu_b[:], pmu[:])
        mu2 = work.tile([128, L], bf16, tag="mu2s", name=f"mu2_{name}", bufs=2)
        nc.vector.tensor_tensor(mu2[:], mu_b[:], mu_b[:], MUL)
        var = work.tile([128, L], f32, tag="vars", name=f"var_{name}", bufs=2)
        nc.vector.tensor_tensor(var[:], psq[:], mu2[:], SUB)
        lnv = work.tile([128, L], f32, tag="vars", name=f"lnv_{name}", bufs=2)
        nc.scalar.activation(lnv[:], var[:], AF.Ln, bias=eps_col[:, 0:1])
        rstd_b = work.tile([128, L], bf16, tag=f"rstdb_{name}", name=f"rstdb_{name}")
        nc.scalar.activation(rstd_b[:], lnv[:], AF.Exp, scale=-0.5)
        outs = []
        for cc in range(2):
            t1 = work.tile([128, L], bf16, tag="hscr", name=f"hs1_{name}{cc}", bufs=2)
            nc.vector.tensor_tensor(t1[:], xTt[:, cc * L:(cc + 1) * L], mu_b[:], SUB)
            hb = work.tile([128, L], bf16, tag=f"hTb_{name}{cc}", name=f"hTb_{name}{cc}")
            nc.vector.tensor_tensor(hb[:], t1[:], rstd_b[:], MUL)
            outs.append(hb)
        return outs

    hT_pre = ln_affine_T(xTpre_t, None, None, "p")
    hT_row = ln_affine_T(xTrow_t, None, None, "r")

    pwarm = pp.tile([128, 512], f32, tag="pwarm", name="pwarm", bufs=1)
    for w in range(24):
        nc.tensor.matmul(pwarm[:], oD256[:], xTpre_t[:, 0:512],
                         start=True, stop=True)



    # ---------------- S3: xi projection -----------------------------------
    def proj_psum(hT, Wt, name):
        outs = []
        for j in range(4):
            ppm = pp.tile([128, L], f32, tag="projp", name=f"pp_{name}{j}", bufs=2)
            for kk in range(2):
                for th in range(2):
                    nc.tensor.matmul(
                        ppm[:, th * 512:(th + 1) * 512],
                        Wt[:, kk * DI + j * 128:kk * DI + (j + 1) * 128],
                        hT[kk][:, th * 512:(th + 1) * 512],
                        start=(kk == 0), stop=(kk == 1))
            yield ppm

    xiT = []
    for j, ppm in enumerate(proj_psum(hT_pre, Wxi_t, "xiT")):
        ot = work.tile([128, L], bf16, tag=f"xiT{j}", name=f"xiT{j}")
        nc.scalar.activation(ot[:], ppm[:], AF.Identity, bias=sml("b_in_xi", j))
        xiT.append(ot)

    # ---------------- S4: depthwise conv 3x3 + silu (PE diagonal) ---------
    xsT = []
    for j in range(4):
        pad = work.tile([128, 34 * 34], bf16, tag="pad", name=f"pad{j}", bufs=4)
        nc.gpsimd.memset(pad[:], 0.0)
        pad3 = pad[:, :].rearrange("p (H W) -> p H W", H=34, W=34)
        src = xiT[j][:, :].rearrange("p (h w) -> p h w", h=32, w=32)
        nc.vector.tensor_copy(pad3[:, 1:33, 1:33], src)
        pc = pp.tile([128, L], f32, tag="projp", name=f"pconv{j}", bufs=2)
        for tap in range(9):
            dy, dx = tap // 3, tap % 3
            for th in range(2):
                sh = pad3[:, dy + th * 16:dy + th * 16 + 16, dx:dx + 32]
                nc.tensor.matmul(
                    pc[:, th * 512:(th + 1) * 512],
                    cdiag_t[:, (j * 9 + tap) * 128:(j * 9 + tap + 1) * 128], sh,
                    start=(tap == 0), stop=(tap == 8))
        xs = wp.tile([128, L], bf16, tag=f"xsT{j}", name=f"xsT{j}")
        nc.scalar.activation(xs[:], pc[:], AF.Silu, bias=sml("convb", j))
        xsT.append(xs)

    # ---------------- S5: x_dbl = [B; C] and dt_r -------------------------
    bc_t = wp.tile([128, L], bf16, tag="bc_t", name="bc_t")
    ppbc = pp.tile([128, L], f32, tag="projp", name="ppbc", bufs=2)
    for kk in range(4):
        for th in range(2):
            nc.tensor.matmul(ppbc[:, th * 512:(th + 1) * 512],
                             Wxp_t[:, kk * 144:kk * 144 + 128],
                             xsT[kk][:, th * 512:(th + 1) * 512],
                             start=(kk == 0), stop=(kk == 3))
    nc.scalar.copy(bc_t[:], ppbc[:])
    dtr_t = wp.tile([16, L], bf16, tag="dtr_t", name="dtr_t")
    ppdtr = pp.tile([16, L], f32, tag="ppdtr", name="ppdtr", bufs=1)
    for kk in range(4):
        for th in range(2):
            nc.tensor.matmul(ppdtr[:, th * 512:(th + 1) * 512],
                             Wxp_t[:, kk * 144 + 128:kk * 144 + 144],
                             xsT[kk][:, th * 512:(th + 1) * 512],
                             start=(kk == 0), stop=(kk == 3))
    nc.scalar.copy(dtr_t[:], ppdtr[:])

    # ---------------- row-domain branch (feeds post only) -----------------
    siluz = []
    for j, ppm in enumerate(proj_psum(hT_row, Wz_t, "zTs")):
        sz = persist.tile([128, L], bf16, tag=f"siluz{j}", name=f"siluz{j}")
        nc.scalar.activation(sz[:], ppm[:], AF.Silu, bias=sml("b_in_z", j))
        siluz.append(sz)


    # dt chains are emitted lazily (chain j inside section j-1's slack)
    dtT, wbc = [None] * 4, [None] * 8

    def emit_dt_chain(j, q):
        ppd = argp.tile([128, L], f32, tag="arg", name=f"ppdt{j}")
        for th in range(2):
            nc.tensor.matmul(ppd[:, th * 512:(th + 1) * 512],
                             Wdt[:, j * 128:(j + 1) * 128],
                             dtr_t[:, th * 512:(th + 1) * 512],
                             start=True, stop=True)
        spx = spool.tile([128, L], f32, tag="spx", name=f"spx{j}", bufs=2)
        nc.scalar.activation(spx[:], ppd[:], AF.Exp, bias=sml("dtb", j))
        dt_b = wp.tile([128, L], bf16, tag=f"dtT{j}", name=f"dtT{j}")
        nc.scalar.activation(dt_b[:], spx[:], AF.Ln, bias=1.0)
        dtT[j] = dt_b
        w_b = spool.tile([128, L], bf16, tag="wTtmp", name=f"wT{j}", bufs=2)
        nc.vector.tensor_tensor(w_b[:], dt_b[:], xsT[j][:], MUL)
        for par in range(2):
            g = 2 * j + par
            wb = wp.tile([128, 2 * L], bf16, tag=f"wbc{g % 5}", name=f"wbc{g}")
            wsrc = w_b[par * 64:par * 64 + 64, :]
            for half in range(2):
                q(wb[0:64, half * L:(half + 1) * L], wsrc)
                q(wb[64:128, half * L:(half + 1) * L], wsrc)
            wbc[g] = wb

    # ---------------- S7/S8: scan stage (8 sections of 32) + AllGathers ---
    # Section g = (j = g//2, par = g%2) runs all 32 state-pairs for one
    # 64-channel d-block. Scans run mostly on GPSIMD (Pool: scan @0.60 eff
    # beats TT @0.42), all bf16 muls on DVE (2x perf mode). B/C broadcast
    # tiles load on the sync HWDGE queue (keeps Pool engine clear of SWDGE).
    pre_ps.close()
    prestack.close()
    scan_st = ExitStack()
    argp = scan_st.enter_context(tc.tile_pool(name="argp", bufs=3, space="PSUM"))
    yps = scan_st.enter_context(tc.tile_pool(name="yps", bufs=1, space="PSUM"))
    spool = scan_st.enter_context(tc.tile_pool(name="spool", bufs=2))
    bpool = scan_st.enter_context(tc.tile_pool(name="bpool", bufs=1))
    emit_dt_chain(0, dma)

    def gout(g):
        if g < 6:
            return T["ys_gb3"][g // 2, :, (g % 2) * 64:(g % 2) * 64 + 64, :]
        return T["ys_gbt"][g - 6, :, :, :]

    def emit_collective(r0, rows):
        outs = (T["ys_gb3"][r0 // 128, :, :, :] if rows == 128
                else T["ys_gbt"][(r0 - 384) // 64, :, :, :])
        nc.gpsimd.collective_compute(
            "AllGather", BYP,
            replica_groups=[[0, 1, 2, 3], [4, 5, 6, 7]],
            ins=[T["ys_lb"][r0:r0 + rows, :]],
            outs=[outs],
        )

    Cb = {}

    def load_cb(i):
        t = bpool.tile([128, L], bf16, tag=f"Cb{i}", name=f"Cb{i}")
        dma(t[:], bc_t[64 + 2 * i:64 + 2 * i + 2, :]
            .partition_broadcast(64).rearrange("d n f -> n d f"))
        Cb[i] = t

    RB = 6
    bb_tiles = {}
    bb_seq = [0]

    def load_bb():
        q = bb_seq[0]
        bb_seq[0] += 1
        m = q % 16
        t = bpool.tile([128, 2 * L], bf16, tag=f"Bb{q % RB}", name=f"Bb{q}")
        for half in range(2):
            i = 2 * m + half
            dma(t[:, half * L:(half + 1) * L],
                bc_t[2 * i:2 * i + 2, :]
                .partition_broadcast(64).rearrange("d n f -> n d f"))
        bb_tiles[q] = t

    for i in range(10):
        load_cb(i)
    for _ in range(RB):
        load_bb()

    COLL = {2: (0, 128), 4: (128, 128), 6: (256, 128), 7: (384, 64)}
    it = 0
    for g in range(8):
        j, par = g // 2, g % 2
        ypt = yps.tile([64, L], f32, tag="ypt", name=f"ypt{g}")
        for th in range(2):
            nc.tensor.matmul(ypt[:, th * 512:(th + 1) * 512],
                             dpsel_b[:, g * 64:(g + 1) * 64],
                             xsT[j][:, th * 512:(th + 1) * 512],
                             start=True, stop=False)
        arg = argp.tile([128, L], f32, tag="arg", name=f"arg{g}")
        for th in range(2):
            nc.tensor.matmul(arg[:, th * 512:(th + 1) * 512],
                             sel_t[:, par * 128:(par + 1) * 128],
                             dtT[j][:, th * 512:(th + 1) * 512],
                             start=True, stop=True)
        hbuf = {}
        for idx in range(32 + pipe_depth):
            if idx < 32:
                i = idx
                dA = spool.tile([128, L], bf16, tag="dA", name=f"dA{it}", bufs=3)
                nc.scalar.activation(dA[:], arg[:], AF.Exp,
                                     scale=acols[:, g * 32 + i:g * 32 + i + 1])
                if i % 2 == 0:
                    xin2 = spool.tile([128, 2 * L], bf16, tag="xin",
                                      name=f"xin{it}", bufs=2)
                    nc.vector.tensor_tensor(xin2[:], wbc[g][:],
                                            bb_tiles[g * 16 + i // 2][:], MUL)
                    cur_xin = xin2
                h = spool.tile([128, L], bf16, tag="h", name=f"h{it}",
                               bufs=pipe_depth + 3)
                nc.vector.tensor_tensor_scan(
                    h[:], dA[:], cur_xin[:, (i % 2) * L:(i % 2 + 1) * L],
                    0.0, MUL, ADD)
                hbuf[idx] = (h, i)
                it += 1
                if i % 2 == 1 and bb_seq[0] < 128:
                    load_bb()
                if g == 0 and 0 <= idx < 22:
                    load_cb(idx + 10)
            if idx >= pipe_depth:
                h, i = hbuf.pop(idx - pipe_depth)
                yc = spool.tile([128, L], bf16, tag="yc",
                                name=f"yc{g}_{i}", bufs=3)
                yeng = nc.vector if (idx % ycmod == 3) else nc.gpsimd
                yeng.tensor_tensor(yc[:], h[:], Cb[i][:], MUL)
                for th in range(2):
                    nc.tensor.matmul(ypt[:, th * 512:(th + 1) * 512],
                                     ysel_b[:], yc[:, th * 512:(th + 1) * 512],
                                     start=False, stop=(i == 31))
            if idx == 4 and g in COLL:
                emit_collective(*COLL[g])
            if idx == 20 and g in (0, 2, 4):
                emit_dt_chain(g // 2 + 1, dma)
        ys16 = spool.tile([64, L], bf16, tag="ys16", name=f"ys16_{g}", bufs=2)
        nc.scalar.copy(ys16[:], ypt[:])
        dma(T["ys_lb"][g * 64:(g + 1) * 64, :], ys16[:])
    emit_collective(448, 64)

    scan_st.close()
    wstack.close()

    post = ExitStack()
    pf = post.enter_context(tc.tile_pool(name="postf", bufs=1))
    ppost = post.enter_context(tc.tile_pool(name="ps_post", bufs=2, space="PSUM"))

    xTr_t = pf.tile([128, 2 * L], bf16, tag="xTr", name="xTr")
    dma_act(xTr_t[:], T["xT_row_b"][:, :])
    Wl_t = load1("W_out", 4 * DIM, pf, q=dma_act)
    Wb_t = load1("W_outb", 4 * DIM, pf, q=dma_act)
    Wfc1g = load1("W_fc1", 2 * HID, pf, q=dma_act)
    Wfc2_t = load1("W_fc2", 8 * DIM, pf, q=dma_act)

    # T2 = Wb^T @ siluz, T3 = Wl^T @ siluz (collective-independent)
    T2b, T3b = [], []
    for cc in range(2):
        p2 = ppost.tile([128, L], f32, tag="pbig", name=f"pT2_{cc}", bufs=2)
        for kk in range(4):
            for th in range(2):
                nc.tensor.matmul(p2[:, th * 512:(th + 1) * 512],
                                 Wb_t[:, kk * DIM + cc * 128:kk * DIM + (cc + 1) * 128],
                                 siluz[kk][:, th * 512:(th + 1) * 512],
                                 start=(kk == 0), stop=(kk == 3))
        t2g = pf.tile([128, L], bf16, tag=f"T2b{cc}", name=f"T2b{cc}")
        nc.scalar.activation(t2g[:], p2[:], AF.Identity,
                             bias=spk[:, 52 + cc:52 + cc + 1],
                             scale=spk[:, 40 + cc:40 + cc + 1])
        xr2 = pf.tile([128, L], bf16, tag=f"xTr2{cc}", name=f"xTr2{cc}")
        nc.vector.tensor_tensor(xr2[:], t2g[:], xTr_t[:, cc * L:(cc + 1) * L], ADD)
        T2b.append(xr2)
    for cc in range(2):
        p3 = ppost.tile([128, L], f32, tag="pbig", name=f"pT3_{cc}", bufs=2)
        for kk in range(4):
            for th in range(2):
                nc.tensor.matmul(p3[:, th * 512:(th + 1) * 512],
                                 Wl_t[:, kk * DIM + cc * 128:kk * DIM + (cc + 1) * 128],
                                 siluz[kk][:, th * 512:(th + 1) * 512],
                                 start=(kk == 0), stop=(kk == 3))
        t3b = pf.tile([128, L], bf16, tag=f"T3b{cc}", name=f"T3b{cc}")
        nc.scalar.copy(t3b[:], p3[:])
        T3b.append(t3b)

    # ---------------- combine directions per j as collectives land --------
    pmu_y = ppost.tile([128, L], f32, tag="pstat", name="pmu_y", bufs=2)
    psq_y = ppost.tile([128, L], f32, tag="pstat", name="psq_y", bufs=2)
    pT1 = [ppost.tile([128, L], f32, tag="pbig", name=f"pT1_{cc}", bufs=2)
           for cc in range(2)]
    for j in range(4):
        ysk_t = pf.tile([128, 4 * L], bf16, tag="ysk", name=f"ysk{j}", bufs=2)
        for par in range(2):
            dma(ysk_t[par * 64:par * 64 + 64, :].rearrange("p (k f) -> p k f", k=4),
                gout(2 * j + par).rearrange("k p f -> p k f"))

        def yv(k):
            return ysk_t[:, k * L:(k + 1) * L]

        rev3 = pf.tile([128, L], bf16, tag="rev3", name=f"rev3_{j}", bufs=2)
        nc.vector.tensor_copy(rev3[:], yv(3)[:, ::-1])
        acc = pf.tile([128, L], bf16, tag="yrow", name=f"yrow{j}_0", bufs=8)
        nc.vector.tensor_tensor(acc[:], yv(0)[:, :], yv(2)[:, ::-1], ADD)
        for k in (1, 3):
            nacc = pf.tile([128, L], bf16, tag="yrow", name=f"yrow{j}_{k}", bufs=8)
            srct = yv(1) if k == 1 else rev3[:, :]
            view = (srct.rearrange("p (w h) -> p w h", w=32, h=32)
                         .rearrange("p w h -> p h w"))
            ceng = nc.gpsimd if (j % 2 == 0) else nc.vector
            ceng.tensor_tensor(
                nacc[:].rearrange("p (h w) -> p h w", h=32, w=32),
                acc[:].rearrange("p (h w) -> p h w", h=32, w=32),
                view, ADD)
            acc = nacc
        ysq = pf.tile([128, L], bf16, tag="ysq", name=f"ysq{j}", bufs=2)
        nc.gpsimd.tensor_tensor(ysq[:], acc[:], acc[:], MUL)
        ysz = pf.tile([128, L], bf16, tag="ysz", name=f"ysz{j}", bufs=2)
        nc.vector.tensor_tensor(ysz[:], acc[:], siluz[j][:], MUL)
        for th in range(2):
            nc.tensor.matmul(pmu_y[:, th * 512:(th + 1) * 512], oD512[:],
                             acc[:, th * 512:(th + 1) * 512],
                             start=(j == 0), stop=(j == 3))
            nc.tensor.matmul(psq_y[:, th * 512:(th + 1) * 512], oD512[:],
                             ysq[:, th * 512:(th + 1) * 512],
                             start=(j == 0), stop=(j == 3))
            for cc in range(2):
                nc.tensor.matmul(
                    pT1[cc][:, th * 512:(th + 1) * 512],
                    Wl_t[:, j * DIM + cc * 128:j * DIM + (cc + 1) * 128],
                    ysz[:, th * 512:(th + 1) * 512],
                    start=(j == 0), stop=(j == 3))

    # ---------------- hy = rstd*T1 - (mu*rstd)*T3 + T2; x2 = x + g*hy -----
    ymu_b = pf.tile([128, L], bf16, tag="ymu_b", name="ymu_b")
    nc.vector.tensor_copy(ymu_b[:], pmu_y[:])
    ymu2 = pf.tile([128, L], bf16, tag="psc", name="ymu2", bufs=4)
    nc.scalar.activation(ymu2[:], pmu_y[:], AF.Square)
    yvar = pf.tile([128, L], f32, tag="pvarf", name="yvar", bufs=2)
    nc.vector.tensor_tensor(yvar[:], psq_y[:], ymu2[:], SUB)
    ylnv = pf.tile([128, L], f32, tag="pvarf", name="ylnv", bufs=2)
    nc.scalar.activation(ylnv[:], yvar[:], AF.Ln, bias=eps_col[:, 0:1])
    yrstd_b = pf.tile([128, L], bf16, tag="yrstd_b", name="yrstd_b")
    nc.scalar.activation(yrstd_b[:], ylnv[:], AF.Exp, scale=-0.5)
    ymr = pf.tile([128, L], bf16, tag="ymr", name="ymr")
    nc.vector.tensor_tensor(ymr[:], ymu_b[:], yrstd_b[:], MUL)

    # keep PE hot through the pointwise hy window so LN2-stats/fc1/fc2 run
    # at full pstate (junk matmuls; pstat ring slots are WAR-safe here)
    for w2 in range(2):
        pw2 = ppost.tile([128, L], f32, tag="pstat", name=f"pw2_{w2}", bufs=2)
        for w in range(14):
            nc.tensor.matmul(pw2[:, 0:512], oD256[:], xTr_t[:, 0:512],
                             start=True, stop=True)

    x2b = []
    for cc in range(2):
        q1 = pf.tile([128, L], bf16, tag="psc", name=f"q1_{cc}", bufs=4)
        nc.vector.tensor_tensor(q1[:], pT1[cc][:], yrstd_b[:], MUL)
        q2 = pf.tile([128, L], bf16, tag="psc", name=f"q2_{cc}", bufs=4)
        nc.vector.tensor_tensor(q2[:], T3b[cc][:], ymr[:], MUL)
        q3 = pf.tile([128, L], bf16, tag="psc", name=f"q3_{cc}", bufs=4)
        nc.vector.tensor_tensor(q3[:], q1[:], q2[:], SUB)
        hyg = pf.tile([128, L], bf16, tag="psc", name=f"hyg{cc}", bufs=4)
        nc.scalar.activation(hyg[:], q3[:], AF.Identity,
                             scale=spk[:, 40 + cc:40 + cc + 1])
        x2 = pf.tile([128, L], bf16, tag=f"x2b{cc}", name=f"x2b{cc}")
        nc.vector.tensor_tensor(x2[:], hyg[:], T2b[cc][:], ADD)
        x2b.append(x2)

    # ---------------- LN2 + modulate (PE ones-broadcast) ------------------
    pmu2s = ppost.tile([128, L], f32, tag="pstat", name="pmu_x2", bufs=2)
    psq2s = ppost.tile([128, L], f32, tag="pstat", name="psq_x2", bufs=2)
    for cc in range(2):
        sq = pf.tile([128, L], bf16, tag="ysq", name=f"sq_x2{cc}", bufs=2)
        qeng = nc.gpsimd if cc == 0 else nc.vector
        qeng.tensor_tensor(sq[:], x2b[cc][:], x2b[cc][:], MUL)
        for th in range(2):
            nc.tensor.matmul(pmu2s[:, th * 512:(th + 1) * 512], oD256[:],
                             x2b[cc][:, th * 512:(th + 1) * 512],
                             start=(cc == 0), stop=(cc == 1))
            nc.tensor.matmul(psq2s[:, th * 512:(th + 1) * 512], oD256[:],
                             sq[:, th * 512:(th + 1) * 512],
                             start=(cc == 0), stop=(cc == 1))
    x2mu_b = pf.tile([128, L], bf16, tag="x2mu_b", name="x2mu_b")
    nc.vector.tensor_copy(x2mu_b[:], pmu2s[:])
    x2mu2 = pf.tile([128, L], bf16, tag="psc", name="x2mu2", bufs=4)
    nc.scalar.activation(x2mu2[:], pmu2s[:], AF.Square)
    x2var = pf.tile([128, L], f32, tag="pvarf", name="x2var", bufs=2)
    nc.vector.tensor_tensor(x2var[:], psq2s[:], x2mu2[:], SUB)
    x2lnv = pf.tile([128, L], f32, tag="pvarf", name="x2lnv", bufs=2)
    nc.scalar.activation(x2lnv[:], x2var[:], AF.Ln, bias=eps_col[:, 0:1])
    x2rstd_b = pf.tile([128, L], bf16, tag="x2rstd_b", name="x2rstd_b")
    nc.scalar.activation(x2rstd_b[:], x2lnv[:], AF.Exp, scale=-0.5)

    mT = []
    for cc in range(2):
        t1 = pf.tile([128, L], bf16, tag="psc", name=f"m1_{cc}", bufs=4)
        nc.vector.tensor_tensor(t1[:], x2b[cc][:], x2mu_b[:], SUB)
        mb = pf.tile([128, L], bf16, tag=f"mT{cc}", name=f"mT{cc}")
        nc.vector.tensor_tensor(mb[:], t1[:], x2rstd_b[:], MUL)
        mT.append(mb)

    # ---------------- MLP + final residual --------------------------------
    gelu = []
    for j in range(8):
        pfc = ppost.tile([128, L], f32, tag="pbig", name=f"pfc1_{j}", bufs=2)
        for kk in range(2):
            for th in range(2):
                nc.tensor.matmul(pfc[:, th * 512:(th + 1) * 512],
                                 Wfc1g[:, kk * HID + j * 128:kk * HID + (j + 1) * 128],
                                 mT[kk][:, th * 512:(th + 1) * 512],
                                 start=(kk == 0), stop=(kk == 1))
        gl = pf.tile([128, L], bf16, tag=f"gelu{j}", name=f"gelu{j}")
        nc.scalar.activation(gl[:], pfc[:], AF.Gelu_apprx_tanh,
                             bias=sml("b_fc1_c", j))
        gelu.append(gl)

    for cc in range(2):
        pfc2 = ppost.tile([128, L], f32, tag="pbig", name=f"pfc2_{cc}", bufs=2)
        for th in range(2):
            for kk in range(8):
                nc.tensor.matmul(pfc2[:, th * 512:(th + 1) * 512],
                                 Wfc2_t[:, kk * DIM + cc * 128:kk * DIM + (cc + 1) * 128],
                                 gelu[kk][:, th * 512:(th + 1) * 512],
                                 start=(kk == 0), stop=(kk == 7))
        t1 = pf.tile([128, L], bf16, tag="psc", name=f"mlpg{cc}", bufs=4)
        nc.scalar.activation(t1[:], pfc2[:], AF.Identity,
                             bias=spk[:, 54 + cc:54 + cc + 1], scale=spk[:, 46 + cc:46 + cc + 1])
        o = pf.tile([128, L], bf16, tag="outTt", name=f"outT{cc}", bufs=2)
        nc.vector.tensor_tensor(o[:], t1[:], x2b[cc][:], ADD)
        dma(T["outT"][cc * 128:(cc + 1) * 128, :], o[:])

    post.close()
    perstack.close()


# ---------------------------------------------------------------------------
# Host side
_PROGRAM = None


def _get_program():
    global _PROGRAM
    if _PROGRAM is None:
        _PROGRAM = build_program()
    return _PROGRAM


def _q_img(x, k):
    img = x.reshape(Hs, Ws, -1)
    if k == 0:
        out = img
    elif k == 1:
        out = img.transpose(1, 0, 2)
    elif k == 2:
        out = img[::-1, ::-1]
    else:
        out = img.transpose(1, 0, 2)[::-1, ::-1]
    return np.ascontiguousarray(out.reshape(L, -1))


def _conv_w_q(w, k):
    if k == 0:
        return w
    if k == 1:
        return np.ascontiguousarray(w.transpose(1, 0, 2))
    if k == 2:
        return np.ascontiguousarray(w[::-1, ::-1])
    return np.ascontiguousarray(w.transpose(1, 0, 2)[::-1, ::-1])


def _col128(v, ncols):
    return np.ascontiguousarray(v.reshape(ncols, 128).T)


def _bf(x):
    import ml_dtypes
    return np.ascontiguousarray(np.asarray(x, np.float32)).astype(ml_dtypes.bfloat16)


def _pack(x, rows=128):
    """(n*rows, cols) -> (rows, n*cols): tile row-blocks side by side."""
    x = np.asarray(x)
    n = x.shape[0] // rows
    return np.ascontiguousarray(
        x.reshape(n, rows, x.shape[1]).transpose(1, 0, 2).reshape(rows, -1))


def prep_inputs(inputs):
    inp = {k: np.asarray(v, dtype=np.float32) for k, v in inputs.items()}
    x, c = inp["x"], inp["c"]

    def silu(v):
        return v / (1.0 + np.exp(-v))

    # host-computed adaLN modulation per sample
    mod = silu(c) @ inp["W_ada"] + inp["b_ada"][None, :]      # (B, 6*DIM)

    shared = {}
    W_in = inp["W_in"]
    shared["W_out"] = _pack(_bf(inp["W_out"] * inp["ln_w"][:, None]))
    shared["W_outb"] = _pack(_bf(inp["W_out"] * inp["ln_b"][:, None]))
    shared["W_fc2"] = _pack(_bf(inp["W_fc2"]))
    p = np.arange(128)
    sel2 = np.zeros((2, 128, 128), np.float32)
    for par in range(2):
        sel2[par, p % 64 + par * 64, p] = -1.0
    shared["sel2n"] = _pack(_bf(sel2.reshape(256, 128)))
    ys = np.zeros((128, 64), np.float32)
    ys[p, p % 64] = 1.0
    shared["ysel"] = _bf(ys)

    sp0 = np.zeros((128, 56), np.float32)
    b_in = inp["b_in"]
    sp0[:, 0:4] = _col128(b_in[:DI], 4)
    sp0[:, 4:8] = _col128(b_in[DI:], 4)
    sp0[:, 8:12] = _col128(inp["conv_b"], 4)
    sp0[:, 16:20] = _col128(inp["ln_w"], 4)
    sp0[:, 20:24] = _col128(inp["ln_b"], 4)

    in_maps = []
    for core in range(8):
        b, k = core // 4, core % 4
        m = dict(shared)
        xb = x[b]
        xpre = _q_img(xb, k)
        m["xT_pre_b"] = _pack(_bf(xpre.T))
        m["xT_row_b"] = _pack(_bf(xb.T))

        mb = mod[b]
        sh_msa, sc_msa, g_msa, sh_mlp, sc_mlp, g_mlp = np.split(mb, 6)
        s1_mlp = 1.0 + sc_mlp
        s1_msa = 1.0 + sc_msa
        m["W_fc1"] = _pack(_bf(inp["W_fc1"] * s1_mlp[:, None]))
        m["W_in_xi"] = _pack(_bf(W_in[:, :DI] * s1_msa[:, None]))
        m["W_in_z"] = _pack(_bf(W_in[:, DI:] * s1_msa[:, None]))

        spc = sp0.copy()
        spc[:, 0:4] = _col128(b_in[:DI] + W_in[:, :DI].T @ sh_msa, 4)
        spc[:, 4:8] = _col128(b_in[DI:] + W_in[:, DI:].T @ sh_msa, 4)
        spc[:, 12:16] = _col128(inp["dt_bias"][k], 4)
        spc[:, 26:34] = _col128(inp["b_fc1"] + inp["W_fc1"].T @ sh_mlp, 8)
        for i6, v in enumerate((sh_msa, sc_msa, g_msa, sh_mlp, sc_mlp, g_mlp)):
            spc[:, 36 + 2 * i6:38 + 2 * i6] = _col128(v, 2)
        spc[:, 48:50] = _col128(1.0 + sc_msa, 2)
        spc[:, 50:52] = _col128(s1_mlp, 2)
        spc[:, 52:54] = _col128(g_msa * inp["b_out"], 2)
        spc[:, 54:56] = _col128(g_mlp * inp["b_fc2"], 2)
        m["smallpack"] = spc

        cw = _conv_w_q(inp["conv_w"].reshape(3, 3, DI), k).reshape(9, DI)
        cd = np.zeros((36, 128, 128), np.float32)
        for j in range(4):
            for tap in range(9):
                np.fill_diagonal(cd[j * 9 + tap], cw[tap, j * 128:(j + 1) * 128])
        m["convdiag"] = _pack(_bf(cd.reshape(36 * 128, 128)))

        Wxp = inp["W_xproj"][k]                           # (DI, 144) cols [dtr,B,C]
        Wxp_r = np.concatenate([Wxp[:, DTR:DTR + DS], Wxp[:, DTR + DS:],
                                Wxp[:, :DTR]], axis=1)
        m["W_xp"] = _pack(_bf(Wxp_r))
        m["W_dtm"] = _bf(inp["W_dt"][k])

        dps = np.zeros((128, 512), np.float32)
        Dpk = inp["Dp"][k]
        for g in range(8):
            par = g % 2
            for po in range(64):
                dps[par * 64 + po, g * 64 + po] = Dpk[g * 64 + po]
        m["dpsel"] = _bf(dps)

        alog = inp["A_log"][k]                            # (DI, DS)
        acols = np.zeros((128, 256), np.float32)
        for g in range(8):
            for i in range(NPAIRS):
                acols[:, g * 32 + i] = np.exp(alog[g * 64 + (p % 64), 2 * i + (p // 64)])
        m["acols"] = acols
        in_maps.append(m)
    return in_maps


def kernel(**inputs):
    nc = _get_program()
    in_maps = prep_inputs(inputs)
    res = run_bass_kernel_spmd(nc, in_maps, list(range(8)))
    out = np.zeros((B, L, DIM), np.float32)
    for b in range(B):
        out[b] = np.asarray(res.results[4 * b]["outT"],
                            dtype=np.float32).T
    return out



# revision 25
# speedup vs baseline: 1.0523x; 1.0523x over previous
"""DiM block (Mamba-style selective-scan transformer block) on 8 TRN2 cores.

Sharding: core i handles (b = i//4, k = i%4). Spatial permutation q_k is
host-prepared so ONE SPMD program serves all 8 cores. Per-sample combine
over k via 4 chunked AllGathers (one per 128-row d-block of ys), overlapped
with the scan (d-block-outer loops, n-range split in two halves so the B/C
row-broadcast tiles load once per half).

Scan tile: partitions = (n-pair:2, d:64), free = t. Per (j, n-half)
section: PE broadcasts -dt into pinned PSUM (selector matmul), ACT applies
exp(A*dt) via per-partition A columns, DVE runs both elementwise muls (bf16
2x), the hw scan runs 7/8 on GPSIMD / 1/8 on DVE, PE reduces n-pairs into a
per-j PSUM accumulator seeded with Dp*xs (host-scaled selector).

LN stats everywhere use PE ones-matmuls on transposed tiles to produce
(128,L) per-token broadcast tiles directly (no transpose/DMA-broadcast
chains). The post phase folds LN-y into three Wout passes (T1 = Wl@(y*sz),
T3 = Wl@sz, T2 = Wlnb@sz) so only pointwise work remains after the last
collective. All weights arrive host-packed bf16, one DMA each.
"""
import json
import sys

sys.path.insert(0, "/opt/trn_rl_repo")

import numpy as np
import concourse.bass as bass
import concourse.mybir as mybir
import concourse.tile as tile
from concourse.bass_utils import run_bass_kernel_spmd

# ---------------------------------------------------------------------------
# Workaround: this walrus build rejects instructions carrying >1 embedded
# sem-wait. Split extra waits onto same-engine NoOps at BIR serialization.
_MAXW = 1
_wsplit_counter = [0]


def _split_multi_waits(bir: dict) -> dict:
    for fn in bir.get("functions", []):
        for bb in fn.get("blocks", []):
            insts = bb.get("instructions", [])
            if not any(
                len((i.get("sync_info") or {}).get("on_wait") or []) > _MAXW
                for i in insts
            ):
                continue
            out = []
            for inst in insts:
                si = inst.get("sync_info")
                waits = (si or {}).get("on_wait") or []
                if len(waits) > _MAXW and inst.get("engine"):
                    for w in waits[:-_MAXW]:
                        _wsplit_counter[0] += 1
                        out.append({
                            "debug": inst.get("debug", 0),
                            "engine": inst["engine"],
                            "ins": [], "outs": [],
                            "name": f"I-wsplit-{_wsplit_counter[0]}",
                            "opcode": "NoOp",
                            "sync_info": {"on_update": [], "on_wait": [w]},
                        })
                    si["on_wait"] = waits[-_MAXW:]
                out.append(inst)
            bb["instructions"] = out
    return bir


_orig_to_json_bytes = bass.Bass.to_json_bytes


def _patched_to_json_bytes(self) -> bytes:
    j = json.loads(_orig_to_json_bytes(self))
    _split_multi_waits(j)
    return json.dumps(j).encode()


bass.Bass.to_json_bytes = _patched_to_json_bytes

# ---------------------------------------------------------------------------
B, Hs, Ws, DIM = 2, 32, 32, 256
L = Hs * Ws
DI = 2 * DIM
DS = 64
DTR = DIM // 16
K = 4
HID = 4 * DIM

f32 = mybir.dt.float32
bf16 = mybir.dt.bfloat16
MUL = mybir.AluOpType.mult
ADD = mybir.AluOpType.add
SUB = mybir.AluOpType.subtract
BYP = mybir.AluOpType.bypass
AF = mybir.ActivationFunctionType
AX = mybir.AxisListType

EPS = 1e-6
NPAIRS = DS // 2

SP_OFF = {"b_in_xi": 0, "b_in_z": 4, "convb": 8, "dtb": 12,
          "lnw_c": 16, "lnb_c": 20, "b_out_c": 24, "b_fc1_c": 26,
          "b_fc2_c": 34}


def build_program(scan_dve_mod=6, pipe_depth=2, ycmod=7):
    nc = bass.Bass()

    def din(name, shape, dt=f32):
        return nc.dram_tensor(name, list(shape), dt, kind="ExternalInput")

    T = {}
    T["xT_pre_b"] = din("xT_pre_b", (128, 2 * L), bf16)
    T["xT_row_b"] = din("xT_row_b", (128, 2 * L), bf16)
    T["W_in_xi"] = din("W_in_xi", (128, 2 * DI), bf16)
    T["W_in_z"] = din("W_in_z", (128, 2 * DI), bf16)
    T["convdiag"] = din("convdiag", (128, 36 * 128), bf16)
    T["W_xp"] = din("W_xp", (128, 4 * 144), bf16)  # cols [B(64), C(64), dtr]
    T["W_dtm"] = din("W_dtm", (DTR, DI), bf16)
    T["acols"] = din("acols", (128, 256))
    T["sel2n"] = din("sel2n", (128, 2 * 128), bf16)
    T["ysel"] = din("ysel", (128, 64), bf16)
    T["dpsel"] = din("dpsel", (128, 512), bf16)
    T["W_out"] = din("W_out", (128, 4 * DIM), bf16)
    T["W_outb"] = din("W_outb", (128, 4 * DIM), bf16)
    T["W_fc1"] = din("W_fc1", (128, 2 * HID), bf16)
    T["W_fc2"] = din("W_fc2", (128, 8 * DIM), bf16)
    T["smallpack"] = din("smallpack", (128, 56))

    T["outT"] = nc.dram_tensor("outT", [DIM, L], bf16, kind="ExternalOutput")
    T["ys_lb"] = nc.dram_tensor("ys_lb", [DI, L], bf16)
    # gathered ys: three 128-row chunks + two 64-row chunks (tail-sized)
    T["ys_gb3"] = nc.dram_tensor("ys_gb3", [3, 4, 128, L], bf16)
    T["ys_gbt"] = nc.dram_tensor("ys_gbt", [2, 4, 64, L], bf16)

    with tile.TileContext(nc) as tc:
        _build_body(nc, tc, T, scan_dve_mod, pipe_depth, ycmod)
    return nc


def _build_body(nc, tc, T, scan_dve_mod, pipe_depth, ycmod=6):
    from contextlib import ExitStack

    dma = nc.sync.dma_start
    dma_act = nc.scalar.dma_start
    dma_gps = nc.gpsimd.dma_start

    perstack = ExitStack()
    persist = perstack.enter_context(tc.tile_pool(name="persist", bufs=1))
    wstack = ExitStack()
    wp = wstack.enter_context(tc.tile_pool(name="weights", bufs=1))
    prestack = ExitStack()
    work = prestack.enter_context(tc.tile_pool(name="prew", bufs=1))
    pre_ps = ExitStack()
    pp = pre_ps.enter_context(tc.tile_pool(name="ps_pre", bufs=2, space="PSUM"))

    # ---------------- S0: single-DMA packed loads -------------------------
    def load1(dname, cols, pool, dt=bf16, name=None, q=dma):
        t = pool.tile([128, cols], dt, tag=name or dname, name=name or dname)
        q(t[:], T[dname][:, :])
        return t

    xTpre_t = load1("xT_pre_b", 2 * L, work)
    spk = persist.tile([128, 56], f32, tag="smallpack", name="smallpack")
    dma(spk[:], T["smallpack"][:, :])
    acols = persist.tile([128, 256], f32, tag="acols", name="acols")
    dma(acols[:], T["acols"][:, :])

    Wxi_t = load1("W_in_xi", 2 * DI, work, q=dma_act)
    cdiag_t = load1("convdiag", 36 * 128, work, q=dma_act)
    Wxp_t = load1("W_xp", 4 * 144, work, q=dma_act)
    sel_t = load1("sel2n", 2 * 128, wp, q=dma_act)
    ysel_b = load1("ysel", 64, wp, q=dma_act)
    dpsel_b = load1("dpsel", 512, wp, q=dma_act)
    Wdt = wp.tile([DTR, DI], bf16, tag="Wdt", name="Wdt")
    dma_act(Wdt[:], T["W_dtm"][:, :])

    def sml(nm, i):
        o = SP_OFF[nm]
        return spk[:, o + i:o + i + 1]

    eps_col = persist.tile([128, 1], f32, tag="eps_col", name="eps_col")
    nc.gpsimd.memset(eps_col[:], EPS)
    oD256 = persist.tile([128, 128], bf16, tag="oD256", name="oD256")
    nc.gpsimd.memset(oD256[:], 1.0 / DIM)
    oD512 = persist.tile([128, 128], bf16, tag="oD512", name="oD512")
    nc.gpsimd.memset(oD512[:], 1.0 / DI)

    # adaLN modulation arrives host-computed in smallpack[36:56]
    mcols = [spk[:, 36 + 2 * i6:36 + 2 * i6 + 2] for i6 in range(6)]
    sh_msa, sc_msa, g_msa, sh_mlp, sc_mlp, g_mlp = mcols
    s1_msa = spk[:, 48:50]
    s1_mlp = spk[:, 50:52]
    gb_out = spk[:, 52:54]
    gb_fc2 = spk[:, 54:56]

    # ---------------- S2: LN1 + modulate via PE ones-broadcast ------------
    def ln_affine_T(xTt, name, pp, work, ptag="projp"):
        pmu = pp.tile([128, L], f32, tag=ptag, name=f"pmu_{name}", bufs=2)
        psq = pp.tile([128, L], f32, tag=ptag, name=f"psq_{name}", bufs=2)
        xsq = []
        for cc in range(2):
            sqt = work.tile([128, L], bf16, tag="xsqT", name=f"xsq_{name}{cc}", bufs=2)
            nc.vector.tensor_tensor(sqt[:], xTt[:, cc * L:(cc + 1) * L],
                                    xTt[:, cc * L:(cc + 1) * L], MUL)
            xsq.append(sqt)
        for cc in range(2):
            for th in range(2):
                nc.tensor.matmul(pmu[:, th * 512:(th + 1) * 512], oD256[:],
                                 xTt[:, cc * L + th * 512:cc * L + (th + 1) * 512],
                                 start=(cc == 0), stop=(cc == 1))
                nc.tensor.matmul(psq[:, th * 512:(th + 1) * 512], oD256[:],
                                 xsq[cc][:, th * 512:(th + 1) * 512],
                                 start=(cc == 0), stop=(cc == 1))
        mu_b = work.tile([128, L], bf16, tag=f"mub_{name}", name=f"mub_{name}")
        nc.scalar.copy(mu_b[:], pmu[:])
        mu2 = work.tile([128, L], bf16, tag="mu2s", name=f"mu2_{name}", bufs=2)
        nc.vector.tensor_tensor(mu2[:], mu_b[:], mu_b[:], MUL)
        var = work.tile([128, L], f32, tag="vars", name=f"var_{name}", bufs=2)
        nc.vector.tensor_tensor(var[:], psq[:], mu2[:], SUB)
        lnv = work.tile([128, L], f32, tag="vars", name=f"lnv_{name}", bufs=2)
        nc.scalar.activation(lnv[:], var[:], AF.Ln, bias=eps_col[:, 0:1])
        rstd_b = work.tile([128, L], bf16, tag=f"rstdb_{name}", name=f"rstdb_{name}")
        nc.scalar.activation(rstd_b[:], lnv[:], AF.Exp, scale=-0.5)
        outs = []
        for cc in range(2):
            t1 = work.tile([128, L], bf16, tag="hscr", name=f"hs1_{name}{cc}", bufs=2)
            nc.vector.tensor_tensor(t1[:], xTt[:, cc * L:(cc + 1) * L], mu_b[:], SUB)
            hb = work.tile([128, L], bf16, tag=f"hTb_{name}{cc}", name=f"hTb_{name}{cc}")
            nc.vector.tensor_tensor(hb[:], t1[:], rstd_b[:], MUL)
            outs.append(hb)
        return outs

    hT_pre = ln_affine_T(xTpre_t, "p", pp, work)

    pwarm = pp.tile([128, 512], f32, tag="pwarm", name="pwarm", bufs=1)
    for w in range(24):
        nc.tensor.matmul(pwarm[:], oD256[:], xTpre_t[:, 0:512],
                         start=True, stop=True)



    # ---------------- S3: xi projection -----------------------------------
    def proj_psum(hT, Wt, name, pp=pp, ptag="projp"):
        for j in range(4):
            ppm = pp.tile([128, L], f32, tag=ptag, name=f"pp_{name}{j}", bufs=2)
            for kk in range(2):
                for th in range(2):
                    nc.tensor.matmul(
                        ppm[:, th * 512:(th + 1) * 512],
                        Wt[:, kk * DI + j * 128:kk * DI + (j + 1) * 128],
                        hT[kk][:, th * 512:(th + 1) * 512],
                        start=(kk == 0), stop=(kk == 1))
            yield ppm

    xiT = []
    for j, ppm in enumerate(proj_psum(hT_pre, Wxi_t, "xiT")):
        ot = work.tile([128, L], bf16, tag=f"xiT{j}", name=f"xiT{j}")
        nc.scalar.activation(ot[:], ppm[:], AF.Identity, bias=sml("b_in_xi", j))
        xiT.append(ot)

    # ---------------- S4: depthwise conv 3x3 + silu (PE diagonal) ---------
    xsT = []
    for j in range(4):
        pad = work.tile([128, 34 * 34], bf16, tag="pad", name=f"pad{j}", bufs=4)
        nc.gpsimd.memset(pad[:], 0.0)
        pad3 = pad[:, :].rearrange("p (H W) -> p H W", H=34, W=34)
        src = xiT[j][:, :].rearrange("p (h w) -> p h w", h=32, w=32)
        nc.vector.tensor_copy(pad3[:, 1:33, 1:33], src)
        pc = pp.tile([128, L], f32, tag="projp", name=f"pconv{j}", bufs=2)
        for tap in range(9):
            dy, dx = tap // 3, tap % 3
            for th in range(2):
                sh = pad3[:, dy + th * 16:dy + th * 16 + 16, dx:dx + 32]
                nc.tensor.matmul(
                    pc[:, th * 512:(th + 1) * 512],
                    cdiag_t[:, (j * 9 + tap) * 128:(j * 9 + tap + 1) * 128], sh,
                    start=(tap == 0), stop=(tap == 8))
        xs = wp.tile([128, L], bf16, tag=f"xsT{j}", name=f"xsT{j}")
        nc.scalar.activation(xs[:], pc[:], AF.Silu, bias=sml("convb", j))
        xsT.append(xs)

    # ---------------- S5: x_dbl = [B; C] and dt_r -------------------------
    bc_t = wp.tile([128, L], bf16, tag="bc_t", name="bc_t")
    ppbc = pp.tile([128, L], f32, tag="projp", name="ppbc", bufs=2)
    for kk in range(4):
        for th in range(2):
            nc.tensor.matmul(ppbc[:, th * 512:(th + 1) * 512],
                             Wxp_t[:, kk * 144:kk * 144 + 128],
                             xsT[kk][:, th * 512:(th + 1) * 512],
                             start=(kk == 0), stop=(kk == 3))
    nc.scalar.copy(bc_t[:], ppbc[:])
    dtr_t = wp.tile([16, L], bf16, tag="dtr_t", name="dtr_t")
    ppdtr = pp.tile([16, L], f32, tag="ppdtr", name="ppdtr", bufs=1)
    for kk in range(4):
        for th in range(2):
            nc.tensor.matmul(ppdtr[:, th * 512:(th + 1) * 512],
                             Wxp_t[:, kk * 144 + 128:kk * 144 + 144],
                             xsT[kk][:, th * 512:(th + 1) * 512],
                             start=(kk == 0), stop=(kk == 3))
    nc.scalar.copy(dtr_t[:], ppdtr[:])

    # dt chains are emitted lazily (chain j inside section j-1's slack)
    dtT, wbc = [None] * 4, [None] * 8

    def emit_dt_chain(j, q):
        ppd = argp.tile([128, L], f32, tag="arg", name=f"ppdt{j}")
        for th in range(2):
            nc.tensor.matmul(ppd[:, th * 512:(th + 1) * 512],
                             Wdt[:, j * 128:(j + 1) * 128],
                             dtr_t[:, th * 512:(th + 1) * 512],
                             start=True, stop=True)
        spx = spool.tile([128, L], f32, tag="spx", name=f"spx{j}", bufs=1)
        nc.scalar.activation(spx[:], ppd[:], AF.Exp, bias=sml("dtb", j))
        dt_b = wp.tile([128, L], bf16, tag=f"dtT{j % 2}", name=f"dtT{j}")
        nc.scalar.activation(dt_b[:], spx[:], AF.Ln, bias=1.0)
        dtT[j] = dt_b
        w_b = spool.tile([128, L], bf16, tag="wTtmp", name=f"wT{j}", bufs=2)
        nc.vector.tensor_tensor(w_b[:], dt_b[:], xsT[j][:], MUL)
        for par in range(2):
            g = 2 * j + par
            wb = wp.tile([128, 2 * L], bf16, tag=f"wbc{g % 3}", name=f"wbc{g}")
            wsrc = w_b[par * 64:par * 64 + 64, :]
            for half in range(2):
                q(wb[0:64, half * L:(half + 1) * L], wsrc)
                q(wb[64:128, half * L:(half + 1) * L], wsrc)
            wbc[g] = wb

    # ---------------- S7/S8: scan stage (8 sections of 32) + AllGathers ---
    # Section g = (j = g//2, par = g%2) runs all 32 state-pairs for one
    # 64-channel d-block. Scans run mostly on GPSIMD (Pool: scan @0.60 eff
    # beats TT @0.42), all bf16 muls on DVE (2x perf mode). B/C broadcast
    # tiles load on the sync HWDGE queue (keeps Pool engine clear of SWDGE).
    pre_ps.close()
    prestack.close()
    scan_st = ExitStack()
    argp = scan_st.enter_context(tc.tile_pool(name="argp", bufs=3, space="PSUM"))
    yps = scan_st.enter_context(tc.tile_pool(name="yps", bufs=1, space="PSUM"))
    spool = scan_st.enter_context(tc.tile_pool(name="spool", bufs=2))
    bpool = scan_st.enter_context(tc.tile_pool(name="bpool", bufs=1))
    emit_dt_chain(0, dma)

    def gout(g):
        if g < 6:
            return T["ys_gb3"][g // 2, :, (g % 2) * 64:(g % 2) * 64 + 64, :]
        return T["ys_gbt"][g - 6, :, :, :]

    def emit_collective(r0, rows):
        outs = (T["ys_gb3"][r0 // 128, :, :, :] if rows == 128
                else T["ys_gbt"][(r0 - 384) // 64, :, :, :])
        nc.gpsimd.collective_compute(
            "AllGather", BYP,
            replica_groups=[[0, 1, 2, 3], [4, 5, 6, 7]],
            ins=[T["ys_lb"][r0:r0 + rows, :]],
            outs=[outs],
        )

    # B/C broadcast tiles: fully resident, loaded once on the sync HWDGE
    # queue in first-use order (Bb pair m before Cb 2m, 2m+1).
    Cb = {}
    Bb = {}
    for m in range(16):
        t = bpool.tile([128, 2 * L], bf16, tag=f"Bb{m}", name=f"Bb{m}")
        for half in range(2):
            i = 2 * m + half
            dma(t[:, half * L:(half + 1) * L],
                bc_t[2 * i:2 * i + 2, :]
                .partition_broadcast(64).rearrange("d n f -> n d f"))
        Bb[m] = t
        for i in (2 * m, 2 * m + 1):
            c = bpool.tile([128, L], bf16, tag=f"Cb{i}", name=f"Cb{i}")
            dma(c[:], bc_t[64 + 2 * i:64 + 2 * i + 2, :]
                .partition_broadcast(64).rearrange("d n f -> n d f"))
            Cb[i] = c

    COLL = {2: (0, 128), 4: (128, 128), 6: (256, 128), 7: (384, 64)}
    it = 0
    for g in range(8):
        j, par = g // 2, g % 2
        ypt = yps.tile([64, L], f32, tag="ypt", name=f"ypt{g}")
        for th in range(2):
            nc.tensor.matmul(ypt[:, th * 512:(th + 1) * 512],
                             dpsel_b[:, g * 64:(g + 1) * 64],
                             xsT[j][:, th * 512:(th + 1) * 512],
                             start=True, stop=False)
        arg = argp.tile([128, L], f32, tag="arg", name=f"arg{g}")
        for th in range(2):
            nc.tensor.matmul(arg[:, th * 512:(th + 1) * 512],
                             sel_t[:, par * 128:(par + 1) * 128],
                             dtT[j][:, th * 512:(th + 1) * 512],
                             start=True, stop=True)
        hbuf = {}
        for idx in range(32 + pipe_depth):
            if idx < 32:
                i = idx
                dA = spool.tile([128, L], bf16, tag="dA", name=f"dA{it}", bufs=3)
                nc.scalar.activation(dA[:], arg[:], AF.Exp,
                                     scale=acols[:, g * 32 + i:g * 32 + i + 1])
                if i % 2 == 0:
                    xin2 = spool.tile([128, 2 * L], bf16, tag="xin",
                                      name=f"xin{it}", bufs=2)
                    nc.vector.tensor_tensor(xin2[:], wbc[g][:],
                                            Bb[i // 2][:], MUL)
                    cur_xin = xin2
                h = spool.tile([128, L], bf16, tag="h", name=f"h{it}",
                               bufs=pipe_depth + 4)
                nc.vector.tensor_tensor_scan(
                    h[:], dA[:], cur_xin[:, (i % 2) * L:(i % 2 + 1) * L],
                    0.0, MUL, ADD)
                hbuf[idx] = (h, i)
                it += 1
            if idx >= pipe_depth:
                h, i = hbuf.pop(idx - pipe_depth)
                yc = spool.tile([128, L], bf16, tag="yc",
                                name=f"yc{g}_{i}", bufs=3)
                if idx % ycmod == 3:
                    nc.vector.tensor_tensor(yc[:], h[:], Cb[i][:], MUL)
                else:
                    # STT encoding: (h bypass 0) mult C — same multiply, but
                    # the gpsimd cost table rates TensorScalarPtr at 0.60
                    # impl efficiency vs TensorTensor-mult's 0.42.
                    nc.gpsimd.scalar_tensor_tensor(
                        yc[:], h[:], 0.0, Cb[i][:], BYP, MUL)
                for th in range(2):
                    nc.tensor.matmul(ypt[:, th * 512:(th + 1) * 512],
                                     ysel_b[:], yc[:, th * 512:(th + 1) * 512],
                                     start=False, stop=(i == 31))
            if idx == 6 and g in COLL:
                emit_collective(*COLL[g])
            if idx == 20 and g in (0, 2, 4):
                emit_dt_chain(g // 2 + 1, dma)
        ys16 = spool.tile([64, L], bf16, tag="ys16", name=f"ys16_{g}", bufs=2)
        nc.scalar.copy(ys16[:], ypt[:])
        dma(T["ys_lb"][g * 64:(g + 1) * 64, :], ys16[:])
    emit_collective(448, 64)

    scan_st.close()
    wstack.close()

    post = ExitStack()
    pf = post.enter_context(tc.tile_pool(name="postf", bufs=1))
    ppost = post.enter_context(tc.tile_pool(name="ps_post", bufs=2, space="PSUM"))

    xTr_t = pf.tile([128, 2 * L], bf16, tag="xTr", name="xTr")
    dma_act(xTr_t[:], T["xT_row_b"][:, :])
    Wz_t = load1("W_in_z", 2 * DI, pf, q=dma_act)
    Wl_t = load1("W_out", 4 * DIM, pf, q=dma_act)
    Wb_t = load1("W_outb", 4 * DIM, pf, q=dma_act)
    Wfc1g = load1("W_fc1", 2 * HID, pf, q=dma_act)
    Wfc2_t = load1("W_fc2", 8 * DIM, pf, q=dma_act)

    # row-domain branch (collective-independent): LN(x)^T -> z -> silu(z)
    hT_row = ln_affine_T(xTr_t, "r", ppost, pf, ptag="pbig")
    siluz = []
    for j, ppm in enumerate(proj_psum(hT_row, Wz_t, "zTs", pp=ppost, ptag="pbig")):
        sz = pf.tile([128, L], bf16, tag=f"siluz{j}", name=f"siluz{j}")
        nc.scalar.activation(sz[:], ppm[:], AF.Silu, bias=sml("b_in_z", j))
        siluz.append(sz)

    # T2 = Wb^T @ siluz, T3 = Wl^T @ siluz (collective-independent)
    T2b, T3b = [], []
    for cc in range(2):
        p2 = ppost.tile([128, L], f32, tag="pbig", name=f"pT2_{cc}", bufs=2)
        for kk in range(4):
            for th in range(2):
                nc.tensor.matmul(p2[:, th * 512:(th + 1) * 512],
                                 Wb_t[:, kk * DIM + cc * 128:kk * DIM + (cc + 1) * 128],
                                 siluz[kk][:, th * 512:(th + 1) * 512],
                                 start=(kk == 0), stop=(kk == 3))
        t2g = pf.tile([128, L], bf16, tag=f"T2b{cc}", name=f"T2b{cc}")
        nc.scalar.activation(t2g[:], p2[:], AF.Identity,
                             bias=spk[:, 52 + cc:52 + cc + 1],
                             scale=spk[:, 40 + cc:40 + cc + 1])
        xr2 = pf.tile([128, L], bf16, tag=f"xTr2{cc}", name=f"xTr2{cc}")
        nc.vector.tensor_tensor(xr2[:], t2g[:], xTr_t[:, cc * L:(cc + 1) * L], ADD)
        T2b.append(xr2)
    for cc in range(2):
        p3 = ppost.tile([128, L], f32, tag="pbig", name=f"pT3_{cc}", bufs=2)
        for kk in range(4):
            for th in range(2):
                nc.tensor.matmul(p3[:, th * 512:(th + 1) * 512],
                                 Wl_t[:, kk * DIM + cc * 128:kk * DIM + (cc + 1) * 128],
                                 siluz[kk][:, th * 512:(th + 1) * 512],
                                 start=(kk == 0), stop=(kk == 3))
        t3b = pf.tile([128, L], bf16, tag=f"T3b{cc}", name=f"T3b{cc}")
        nc.scalar.copy(t3b[:], p3[:])
        T3b.append(t3b)

    # ---------------- combine directions per j as collectives land --------
    pmu_y = ppost.tile([128, L], f32, tag="pstat", name="pmu_y", bufs=2)
    psq_y = ppost.tile([128, L], f32, tag="pstat", name="psq_y", bufs=2)
    pT1 = [ppost.tile([128, L], f32, tag="pbig", name=f"pT1_{cc}", bufs=2)
           for cc in range(2)]
    for j in range(4):
        ysk_t = pf.tile([128, 4 * L], bf16, tag="ysk", name=f"ysk{j}", bufs=2)
        for par in range(2):
            dma(ysk_t[par * 64:par * 64 + 64, :].rearrange("p (k f) -> p k f", k=4),
                gout(2 * j + par).rearrange("k p f -> p k f"))

        def yv(k):
            return ysk_t[:, k * L:(k + 1) * L]

        rev3 = pf.tile([128, L], bf16, tag="rev3", name=f"rev3_{j}", bufs=2)
        nc.vector.tensor_copy(rev3[:], yv(3)[:, ::-1])
        acc = pf.tile([128, L], bf16, tag="yrow", name=f"yrow{j}_0", bufs=8)
        nc.vector.tensor_tensor(acc[:], yv(0)[:, :], yv(2)[:, ::-1], ADD)
        for k in (1, 3):
            nacc = pf.tile([128, L], bf16, tag="yrow", name=f"yrow{j}_{k}", bufs=8)
            srct = yv(1) if k == 1 else rev3[:, :]
            view = (srct.rearrange("p (w h) -> p w h", w=32, h=32)
                         .rearrange("p w h -> p h w"))
            ceng = nc.gpsimd if (j % 2 == 0) else nc.vector
            ceng.tensor_tensor(
                nacc[:].rearrange("p (h w) -> p h w", h=32, w=32),
                acc[:].rearrange("p (h w) -> p h w", h=32, w=32),
                view, ADD)
            acc = nacc
        ysq = pf.tile([128, L], bf16, tag="ysq", name=f"ysq{j}", bufs=2)
        nc.gpsimd.tensor_tensor(ysq[:], acc[:], acc[:], MUL)
        ysz = pf.tile([128, L], bf16, tag="ysz", name=f"ysz{j}", bufs=2)
        nc.vector.tensor_tensor(ysz[:], acc[:], siluz[j][:], MUL)
        for th in range(2):
            nc.tensor.matmul(pmu_y[:, th * 512:(th + 1) * 512], oD512[:],
                             acc[:, th * 512:(th + 1) * 512],
                             start=(j == 0), stop=(j == 3))
            nc.tensor.matmul(psq_y[:, th * 512:(th + 1) * 512], oD512[:],
                             ysq[:, th * 512:(th + 1) * 512],
                             start=(j == 0), stop=(j == 3))
            for cc in range(2):
                nc.tensor.matmul(
                    pT1[cc][:, th * 512:(th + 1) * 512],
                    Wl_t[:, j * DIM + cc * 128:j * DIM + (cc + 1) * 128],
                    ysz[:, th * 512:(th + 1) * 512],
                    start=(j == 0), stop=(j == 3))

    # ---------------- hy = rstd*T1 - (mu*rstd)*T3 + T2; x2 = x + g*hy -----
    ymu_b = pf.tile([128, L], bf16, tag="ymu_b", name="ymu_b")
    nc.vector.tensor_copy(ymu_b[:], pmu_y[:])
    ymu2 = pf.tile([128, L], bf16, tag="psc", name="ymu2", bufs=4)
    nc.scalar.activation(ymu2[:], pmu_y[:], AF.Square)
    yvar = pf.tile([128, L], f32, tag="pvarf", name="yvar", bufs=2)
    nc.vector.tensor_tensor(yvar[:], psq_y[:], ymu2[:], SUB)
    ylnv = pf.tile([128, L], f32, tag="pvarf", name="ylnv", bufs=2)
    nc.scalar.activation(ylnv[:], yvar[:], AF.Ln, bias=eps_col[:, 0:1])
    yrstd_b = pf.tile([128, L], bf16, tag="yrstd_b", name="yrstd_b")
    nc.scalar.activation(yrstd_b[:], ylnv[:], AF.Exp, scale=-0.5)
    ymr = pf.tile([128, L], bf16, tag="ymr", name="ymr")
    nc.vector.tensor_tensor(ymr[:], ymu_b[:], yrstd_b[:], MUL)

    # keep PE hot through the pointwise hy window so LN2-stats/fc1/fc2 run
    # at full pstate (junk matmuls; pstat ring slots are WAR-safe here)
    for w2 in range(2):
        pw2 = ppost.tile([128, L], f32, tag="pstat", name=f"pw2_{w2}", bufs=2)
        for w in range(14):
            nc.tensor.matmul(pw2[:, 0:512], oD256[:], xTr_t[:, 0:512],
                             start=True, stop=True)

    x2b = []
    for cc in range(2):
        q1 = pf.tile([128, L], bf16, tag="psc", name=f"q1_{cc}", bufs=4)
        nc.vector.tensor_tensor(q1[:], pT1[cc][:], yrstd_b[:], MUL)
        q2 = pf.tile([128, L], bf16, tag="psc", name=f"q2_{cc}", bufs=4)
        nc.vector.tensor_tensor(q2[:], T3b[cc][:], ymr[:], MUL)
        q3 = pf.tile([128, L], bf16, tag="psc", name=f"q3_{cc}", bufs=4)
        nc.vector.tensor_tensor(q3[:], q1[:], q2[:], SUB)
        hyg = pf.tile([128, L], bf16, tag="psc", name=f"hyg{cc}", bufs=4)
        nc.scalar.activation(hyg[:], q3[:], AF.Identity,
                             scale=spk[:, 40 + cc:40 + cc + 1])
        x2 = pf.tile([128, L], bf16, tag=f"x2b{cc}", name=f"x2b{cc}")
        nc.vector.tensor_tensor(x2[:], hyg[:], T2b[cc][:], ADD)
        x2b.append(x2)

    # ---------------- LN2 + modulate (PE ones-broadcast) ------------------
    pmu2s = ppost.tile([128, L], f32, tag="pstat", name="pmu_x2", bufs=2)
    psq2s = ppost.tile([128, L], f32, tag="pstat", name="psq_x2", bufs=2)
    for cc in range(2):
        sq = pf.tile([128, L], bf16, tag="ysq", name=f"sq_x2{cc}", bufs=2)
        qeng = nc.gpsimd if cc == 0 else nc.vector
        qeng.tensor_tensor(sq[:], x2b[cc][:], x2b[cc][:], MUL)
        for th in range(2):
            nc.tensor.matmul(pmu2s[:, th * 512:(th + 1) * 512], oD256[:],
                             x2b[cc][:, th * 512:(th + 1) * 512],
                             start=(cc == 0), stop=(cc == 1))
            nc.tensor.matmul(psq2s[:, th * 512:(th + 1) * 512], oD256[:],
                             sq[:, th * 512:(th + 1) * 512],
                             start=(cc == 0), stop=(cc == 1))
    x2mu_b = pf.tile([128, L], bf16, tag="x2mu_b", name="x2mu_b")
    nc.vector.tensor_copy(x2mu_b[:], pmu2s[:])
    x2mu2 = pf.tile([128, L], bf16, tag="psc", name="x2mu2", bufs=4)
    nc.scalar.activation(x2mu2[:], pmu2s[:], AF.Square)
    x2var = pf.tile([128, L], f32, tag="pvarf", name="x2var", bufs=2)
    nc.vector.tensor_tensor(x2var[:], psq2s[:], x2mu2[:], SUB)
    x2lnv = pf.tile([128, L], f32, tag="pvarf", name="x2lnv", bufs=2)
    nc.scalar.activation(x2lnv[:], x2var[:], AF.Ln, bias=eps_col[:, 0:1])
    x2rstd_b = pf.tile([128, L], bf16, tag="x2rstd_b", name="x2rstd_b")
    nc.scalar.activation(x2rstd_b[:], x2lnv[:], AF.Exp, scale=-0.5)

    mT = []
    for cc in range(2):
        t1 = pf.tile([128, L], bf16, tag="psc", name=f"m1_{cc}", bufs=4)
        nc.vector.tensor_tensor(t1[:], x2b[cc][:], x2mu_b[:], SUB)
        mb = pf.tile([128, L], bf16, tag=f"mT{cc}", name=f"mT{cc}")
        nc.vector.tensor_tensor(mb[:], t1[:], x2rstd_b[:], MUL)
        mT.append(mb)

    # ---------------- MLP + final residual --------------------------------
    gelu = []
    for j in range(8):
        pfc = ppost.tile([128, L], f32, tag="pbig", name=f"pfc1_{j}", bufs=2)
        for kk in range(2):
            for th in range(2):
                nc.tensor.matmul(pfc[:, th * 512:(th + 1) * 512],
                                 Wfc1g[:, kk * HID + j * 128:kk * HID + (j + 1) * 128],
                                 mT[kk][:, th * 512:(th + 1) * 512],
                                 start=(kk == 0), stop=(kk == 1))
        gl = pf.tile([128, L], bf16, tag=f"gelu{j}", name=f"gelu{j}")
        nc.scalar.activation(gl[:], pfc[:], AF.Gelu_apprx_tanh,
                             bias=sml("b_fc1_c", j))
        gelu.append(gl)

    for cc in range(2):
        pfc2 = ppost.tile([128, L], f32, tag="pbig", name=f"pfc2_{cc}", bufs=2)
        for th in range(2):
            for kk in range(8):
                nc.tensor.matmul(pfc2[:, th * 512:(th + 1) * 512],
                                 Wfc2_t[:, kk * DIM + cc * 128:kk * DIM + (cc + 1) * 128],
                                 gelu[kk][:, th * 512:(th + 1) * 512],
                                 start=(kk == 0), stop=(kk == 7))
        t1 = pf.tile([128, L], bf16, tag="psc", name=f"mlpg{cc}", bufs=4)
        nc.scalar.activation(t1[:], pfc2[:], AF.Identity,
                             bias=spk[:, 54 + cc:54 + cc + 1], scale=spk[:, 46 + cc:46 + cc + 1])
        o = pf.tile([128, L], bf16, tag="outTt", name=f"outT{cc}", bufs=2)
        nc.vector.tensor_tensor(o[:], t1[:], x2b[cc][:], ADD)
        dma(T["outT"][cc * 128:(cc + 1) * 128, :], o[:])

    post.close()
    perstack.close()


# ---------------------------------------------------------------------------
# Host side
_PROGRAM = None


def _get_program():
    global _PROGRAM
    if _PROGRAM is None:
        _PROGRAM = build_program()
    return _PROGRAM


def _q_img(x, k):
    img = x.reshape(Hs, Ws, -1)
    if k == 0:
        out = img
    elif k == 1:
        out = img.transpose(1, 0, 2)
    elif k == 2:
        out = img[::-1, ::-1]
    else:
        out = img.transpose(1, 0, 2)[::-1, ::-1]
    return np.ascontiguousarray(out.reshape(L, -1))


def _conv_w_q(w, k):
    if k == 0:
        return w
    if k == 1:
        return np.ascontiguousarray(w.transpose(1, 0, 2))
    if k == 2:
        return np.ascontiguousarray(w[::-1, ::-1])
    return np.ascontiguousarray(w.transpose(1, 0, 2)[::-1, ::-1])


def _col128(v, ncols):
    return np.ascontiguousarray(v.reshape(ncols, 128).T)


def _bf(x):
    import ml_dtypes
    return np.ascontiguousarray(np.asarray(x, np.float32)).astype(ml_dtypes.bfloat16)


def _pack(x, rows=128):
    """(n*rows, cols) -> (rows, n*cols): tile row-blocks side by side."""
    x = np.asarray(x)
    n = x.shape[0] // rows
    return np.ascontiguousarray(
        x.reshape(n, rows, x.shape[1]).transpose(1, 0, 2).reshape(rows, -1))


def prep_inputs(inputs):
    inp = {k: np.asarray(v, dtype=np.float32) for k, v in inputs.items()}
    x, c = inp["x"], inp["c"]

    def silu(v):
        return v / (1.0 + np.exp(-v))

    # host-computed adaLN modulation per sample
    mod = silu(c) @ inp["W_ada"] + inp["b_ada"][None, :]      # (B, 6*DIM)

    shared = {}
    W_in = inp["W_in"]
    shared["W_out"] = _pack(_bf(inp["W_out"] * inp["ln_w"][:, None]))
    shared["W_outb"] = _pack(_bf(inp["W_out"] * inp["ln_b"][:, None]))
    shared["W_fc2"] = _pack(_bf(inp["W_fc2"]))
    p = np.arange(128)
    sel2 = np.zeros((2, 128, 128), np.float32)
    for par in range(2):
        sel2[par, p % 64 + par * 64, p] = -1.0
    shared["sel2n"] = _pack(_bf(sel2.reshape(256, 128)))
    ys = np.zeros((128, 64), np.float32)
    ys[p, p % 64] = 1.0
    shared["ysel"] = _bf(ys)

    sp0 = np.zeros((128, 56), np.float32)
    b_in = inp["b_in"]
    sp0[:, 0:4] = _col128(b_in[:DI], 4)
    sp0[:, 4:8] = _col128(b_in[DI:], 4)
    sp0[:, 8:12] = _col128(inp["conv_b"], 4)
    sp0[:, 16:20] = _col128(inp["ln_w"], 4)
    sp0[:, 20:24] = _col128(inp["ln_b"], 4)

    in_maps = []
    for core in range(8):
        b, k = core // 4, core % 4
        m = dict(shared)
        xb = x[b]
        xpre = _q_img(xb, k)
        m["xT_pre_b"] = _pack(_bf(xpre.T))
        m["xT_row_b"] = _pack(_bf(xb.T))

        mb = mod[b]
        sh_msa, sc_msa, g_msa, sh_mlp, sc_mlp, g_mlp = np.split(mb, 6)
        s1_mlp = 1.0 + sc_mlp
        s1_msa = 1.0 + sc_msa
        m["W_fc1"] = _pack(_bf(inp["W_fc1"] * s1_mlp[:, None]))
        m["W_in_xi"] = _pack(_bf(W_in[:, :DI] * s1_msa[:, None]))
        m["W_in_z"] = _pack(_bf(W_in[:, DI:] * s1_msa[:, None]))

        spc = sp0.copy()
        spc[:, 0:4] = _col128(b_in[:DI] + W_in[:, :DI].T @ sh_msa, 4)
        spc[:, 4:8] = _col128(b_in[DI:] + W_in[:, DI:].T @ sh_msa, 4)
        spc[:, 12:16] = _col128(inp["dt_bias"][k], 4)
        spc[:, 26:34] = _col128(inp["b_fc1"] + inp["W_fc1"].T @ sh_mlp, 8)
        for i6, v in enumerate((sh_msa, sc_msa, g_msa, sh_mlp, sc_mlp, g_mlp)):
            spc[:, 36 + 2 * i6:38 + 2 * i6] = _col128(v, 2)
        spc[:, 48:50] = _col128(1.0 + sc_msa, 2)
        spc[:, 50:52] = _col128(s1_mlp, 2)
        spc[:, 52:54] = _col128(g_msa * inp["b_out"], 2)
        spc[:, 54:56] = _col128(g_mlp * inp["b_fc2"], 2)
        m["smallpack"] = spc

        cw = _conv_w_q(inp["conv_w"].reshape(3, 3, DI), k).reshape(9, DI)
        cd = np.zeros((36, 128, 128), np.float32)
        for j in range(4):
            for tap in range(9):
                np.fill_diagonal(cd[j * 9 + tap], cw[tap, j * 128:(j + 1) * 128])
        m["convdiag"] = _pack(_bf(cd.reshape(36 * 128, 128)))

        Wxp = inp["W_xproj"][k]                           # (DI, 144) cols [dtr,B,C]
        Wxp_r = np.concatenate([Wxp[:, DTR:DTR + DS], Wxp[:, DTR + DS:],
                                Wxp[:, :DTR]], axis=1)
        m["W_xp"] = _pack(_bf(Wxp_r))
        m["W_dtm"] = _bf(inp["W_dt"][k])

        dps = np.zeros((128, 512), np.float32)
        Dpk = inp["Dp"][k]
        for g in range(8):
            par = g % 2
            for po in range(64):
                dps[par * 64 + po, g * 64 + po] = Dpk[g * 64 + po]
        m["dpsel"] = _bf(dps)

        alog = inp["A_log"][k]                            # (DI, DS)
        acols = np.zeros((128, 256), np.float32)
        for g in range(8):
            for i in range(NPAIRS):
                acols[:, g * 32 + i] = np.exp(alog[g * 64 + (p % 64), 2 * i + (p // 64)])
        m["acols"] = acols
        in_maps.append(m)
    return in_maps


def kernel(**inputs):
    nc = _get_program()
    in_maps = prep_inputs(inputs)
    res = run_bass_kernel_spmd(nc, in_maps, list(range(8)))
    out = np.zeros((B, L, DIM), np.float32)
    for b in range(B):
        out[b] = np.asarray(res.results[4 * b]["outT"],
                            dtype=np.float32).T
    return out

